# revision 61
# baseline (speedup 1.0000x reference)
"""Multi-head attention (B=8, C=512, L=2048, H=8, D=64) on 8 TRN2 NeuronCores.

Sharding: pure batch-parallel - core b computes batch b end-to-end (qkv proj,
8 heads of attention, out proj). No collectives.

Per-core layout strategy (v1 baseline, 357us):
  - qkv projection with lhsT = w_qkv.T (host-transposed), rhs = x.
  - S^T = K^T Q  (keys on partitions) so the exp output is already the
    transposed P^T needed by the PV matmul, and no max-subtraction is needed
    (scores are ~N(0,1) after the 1/sqrt(D) scale, folded into exp's scale).
  - Heads are processed in pairs (2t, 2t+1) that live in partition halves
    0-63 / 64-127 of one qkv row-tile. The two K=64 S^T matmuls of a pair
    run CONCURRENTLY in the PE array (row groups 0-1 vs 2-3) and write the
    two 512-column halves of one [128, 1024] PSUM tile.
  - PV uses lhsT = [V^T | ones] (65 columns): row 64 of the accumulator is
    the softmax denominator, computed for free.
  - V^T is computed directly from X (lhsT = X tiles), V is never materialized.
  - i is processed in 512-wide chunks (outer loop) so each chunk of the
    output projection overlaps the next chunk's attention pass.

v3: ScalarE/VectorE column-split exp (ScalarE table exp on cols 0:CS,
  VectorE 2-op poly16 custom-DVE exp on CS:1024; per-query softmax stays on
  one engine path so per-path bias cancels in the ratio). All matmuls bf16
  (fp8 anywhere on the score/weight path fails the 2e-2 gate). p1 stays f32.

v4-v10 changes (from v3's trace: 421us span, ScalarE ACT 1031ns/jt pacing
a ~1000ns/jt loop, ~3.3us ScalarE idle at each pair boundary, 20us
startup, 24us tail; final: ~343us):
  - Lead-2 S^T: the last two iterations of a pair emit the NEXT pair's
    S^T(0)/S^T(1), so at a boundary the exp chain never waits behind the
    previous pair's last PVs in the in-order PE stream.
  - Deferred normalization: the o2u/den/recip chain is emitted at jt=0 of
    the NEXT pair's loop (both o2u copies first - they free the shared ot
    psum that PV(0) accumulates into), normalize muls at jt=8/9 (earlier
    muls head-of-line block the VectorE queue and the polys behind it).
    The reciprocal row is broadcast to 64 partitions via K=1 ones-row
    matmuls into a qkp psum at jt=3 (v13, -1.4us: the serial ~1.2us
    GpSimd broadcasts gated the second mul through the backlogged VectorE
    recip chain); norms hosted in pass-0 pairs keep GpSimd (no free qkp
    slot there).
  - Per-jt exp split: the first NSC=5 jts of each pair are ALL-ScalarE
    (CS=1024) so VectorE is free for the norm chain + interleaved psum
    evacuations; jts 5..15 use CS2 so the average is KERNEL_CS=896.
    Mixing engines across jt within one query's softmax row only leaves
    the engines' relative approximation error (~2e-4/weight) - harmless
    against the 2e-2 gate (measured: rel err 0.00485 vs 0.00483 uniform).
  - Startup: per-chunk x tiles and per-projection w tiles (DMA-write ->
    read dependency tracking is tile-grained - a monolithic x tile made
    the first k-projection wait for ALL its chunk DMAs); input DMAs spread
    over sync/gpsimd/scalar trigger queues (VectorE cannot trigger DMAs),
    first-needed first; warmup cut to 4 matmuls (real qkv work continues
    the HAM clock ramp); warm psum evacuated via ScalarE (on VectorE it
    head-of-line blocked the first q/k evacuation CASTs).
  - Tail: proj chunk-3 groups become held c0-c2 partials finished after
    the last normalize; the last pair's reciprocal rows are broadcast via
    K=1 ones-row matmuls into a free stp psum (PE is idle at the tail;
    the two serial GpSimd broadcasts were on the critical path), its h1
    norm chain runs on ScalarE, final bias adds on ScalarE, and the last
    out-DMAs go out on three different trigger queues.
  - Engines require 32-aligned partition offsets (ones column stays at
    index 64 with an aligned [64:65] den staging copy; an ones-at-0
    layout is unimplementable). Failed experiments: spreading interleaved
    qk/proj groups one matmul per jt (v9: neutral-to-worse); CS=928 and
    NSC in {3,4,6} (all within noise or worse).
"""

import os
import sys

sys.path.insert(0, "/opt/trn_rl_repo")

import numpy as np
import ml_dtypes

import concourse.bass as bass
import concourse.tile as tile
from concourse import bacc, mybir
from concourse import bass_utils

# ---- custom DVE exp: p = poly4(v), then p^16 (v = 0.125*S/16) -------------
from concourse.dve_spec import Spec, Src0, C0, C1, C2, One, sq, lower, _has_src1
import concourse.dve_ops as dve_ops
from concourse.dve_ops import DveOp
from concourse.dve_uop import DveOpSpec

EXP_C = (0.50053141, 0.16821747, 0.03882078)  # minimax on v in [-0.5125, 0.5125]


def _register_dve_op(name, spec):
    if name in dve_ops._SUB_OPCODE_FOR_NAME:
        return next(op for op in dve_ops.OPS if op.name == name)
    row = max(dve_ops._SUB_OPCODE_FOR_NAME.values()) + 1
    assert row < 0x20
    dve_ops._SUB_OPCODE_FOR_NAME[name] = row
    shas = {}
    for ver in ("v3", "v4"):
        s = DveOpSpec(
            name=name, opcode=row, uops=lower(spec, ver=ver), rd1_en=_has_src1(spec)
        )
        shas[ver] = s.sha(ver)
    op = DveOp(name, spec, subdim=False, uops_sha=shas)
    dve_ops.OPS.append(op)
    dve_ops.CUSTOM_DVE_SPECS[name] = spec
    return op


def _make_exp_ops():
    t = sq(Src0)
    spec1 = Spec(
        body=(One + Src0) + t * (C0 + C1 * Src0 + C2 * t),
        reference=lambda in0, in1, s0, s1, imm2: (
            1.0 + in0 + in0 * in0 * (s0 + s1 * in0 + imm2 * in0 * in0)
        ).astype(np.float32),
    )
    spec2 = Spec(
        body=sq(sq(sq(sq(Src0)))),
        reference=lambda in0, in1, s0, s1, imm2: (in0**16).astype(np.float32),
    )
    return (
        _register_dve_op("EXP16_POLY_ANT", spec1),
        _register_dve_op("POW16_ANT", spec2),
    )


EXP16_POLY, POW16 = _make_exp_ops()

B, C, L = 8, 512, 2048
H, D = 8, 64
HID = H * D  # 512
SCALE = float(D) ** -0.5
BF16 = mybir.dt.bfloat16
F32 = mybir.dt.float32
AF = mybir.ActivationFunctionType
NCORES = 8

NT = C // 128  # 4 channel tiles
NL = L // 512  # 4 l-chunks of 512
NJ = L // 128  # 16 key tiles

# ScalarE's AVERAGE contiguous column share of each [128, 1024] exp tile
# (rest -> VectorE poly). v5: the split varies by jt - the first NSC jts of
# each pair are ALL-ScalarE (the VectorE queue is busy with the previous
# pair's deferred normalization then), and jts NSC..15 use a smaller CS2 so
# the average matches KERNEL_CS. Mixing engines across jt within one
# query's softmax row leaves only the engines' relative approximation
# error (~2e-4 per weight) unconcealed - far inside the 2e-2 gate.
CS = int(os.environ.get("KERNEL_CS", "896"))
NSC = int(os.environ.get("KERNEL_NSC", "5"))  # all-scalar jts per pair
CS2 = min(1024, (16 * CS - NSC * 1024) // (16 - NSC)) if NSC < 16 else 1024
NWARM = int(os.environ.get("KERNEL_NWARM", "4"))


def build_kernel(tc, out_d, x_d, wqkvT_d, woutT_d, bias_d):
    nc = tc.nc
    from contextlib import ExitStack

    ctx = ExitStack()
    pers = ctx.enter_context(tc.tile_pool(name="pers", bufs=1))
    ptp = ctx.enter_context(tc.tile_pool(name="ptp", bufs=6))
    scrp = ctx.enter_context(tc.tile_pool(name="scrp", bufs=3))
    ytp = ctx.enter_context(tc.tile_pool(name="ytp", bufs=4))
    smp = ctx.enter_context(tc.tile_pool(name="smp", bufs=3))  # o2u/rec/rb span a pair boundary
    stp = ctx.enter_context(tc.tile_pool(name="stp", bufs=2, space="PSUM"))
    otp = ctx.enter_context(tc.tile_pool(name="otp", bufs=1, space="PSUM"))
    qkp = ctx.enter_context(tc.tile_pool(name="qkp", bufs=2, space="PSUM"))

    # ---- persistent SBUF tensors. x and wqkv are SPLIT into per-chunk /
    # per-projection tiles: the framework's DMA-write -> engine-read
    # dependency tracking is tile-grained, so a single [128, 2048] x tile
    # written by 4 chunk DMAs made the first k-projection wait for ALL of
    # them (measured +5us on the first-exp critical path). ----
    x_sb = [
        [pers.tile([128, 512], BF16, tag=f"x{c}_{n}", name=f"x{c}_{n}")
         for n in range(NL)]
        for c in range(NT)
    ]
    # q/k weights: the t=0 column block gets its OWN tile so the first
    # q/k projections aren't tile-grain-blocked on the t>=1 column DMAs
    wqq0_sb = [
        pers.tile([128, 128], BF16, tag=f"wqq0{c}", name=f"wqq0{c}") for c in range(NT)
    ]
    wqq_sb = [
        pers.tile([128, 384], BF16, tag=f"wqq{c}", name=f"wqq{c}") for c in range(NT)
    ]
    wqk0_sb = [
        pers.tile([128, 128], BF16, tag=f"wqk0{c}", name=f"wqk0{c}") for c in range(NT)
    ]
    wqk_sb = [
        pers.tile([128, 384], BF16, tag=f"wqk{c}", name=f"wqk{c}") for c in range(NT)
    ]
    wqv_sb = [
        pers.tile([128, HID], BF16, tag=f"wqv{c}", name=f"wqv{c}") for c in range(NT)
    ]
    wo_sb = [pers.tile([128, C], BF16, tag=f"wo{c}", name=f"wo{c}") for c in range(NT)]
    bias_sb = [
        pers.tile([128, 1], F32, tag=f"bias{c}", name=f"bias{c}") for c in range(NT)
    ]
    q_sb = [pers.tile([128, L], BF16, tag=f"q{t}", name=f"q{t}") for t in range(NT)]
    k_sb = [pers.tile([128, L], BF16, tag=f"k{t}", name=f"k{t}") for t in range(NT)]
    vt1 = [
        pers.tile([128, H * 65], BF16, tag=f"vt{j}", name=f"vt{j}") for j in range(NJ)
    ]
    o2 = [pers.tile([128, L], BF16, tag=f"o2_{c}", name=f"o2_{c}") for c in range(NT)]

    warm_scratch = nc.dram_tensor("warm_scratch", [128, 512], F32)
    warm_sb = pers.tile([128, 512], BF16, tag="warm", name="warm_sb")
    warm_out = pers.tile([128, 512], F32, tag="warmo", name="warm_out")
    # warm memset on GpSimd: it is idle at ~6us (VectorE's queue opens
    # later), so the warm-up matmuls start ~1.5us earlier. One extra
    # LOAD_LIB swap, paid once at startup.
    nc.gpsimd.memset(warm_sb[:, :], 0.001)

    # ---- input DMAs spread over 3 trigger queues (VectorE cannot trigger
    # DMAs) so the ~0.7us-per-DMA issue serialization overlaps. First-
    # needed first: pair (0,0) needs x chunk 0 + t=0 q-cols + t=0 k-cols. ----
    # (transfer rate is ~1.1us per 128KB chunk per queue - the x0 chunks
    # that gate the first q/k projections are split across sync+gpsimd)
    rr = [slice(128 * c, 128 * (c + 1)) for c in range(NT)]
    for c in range(NT):
        nc.scalar.dma_start(wqq0_sb[c][:, :], wqkvT_d[rr[c], 0:128])
    nc.sync.dma_start(x_sb[0][0][:, :], x_d[rr[0], 0:512])
    nc.gpsimd.dma_start(x_sb[2][0][:, :], x_d[rr[2], 0:512])
    nc.sync.dma_start(x_sb[1][0][:, :], x_d[rr[1], 0:512])
    nc.gpsimd.dma_start(x_sb[3][0][:, :], x_d[rr[3], 0:512])
    for c in range(NT):
        nc.scalar.dma_start(wqk0_sb[c][:, :], wqkvT_d[rr[c], 512:640])
        nc.sync.dma_start(wqv_sb[c][:, :], wqkvT_d[rr[c], 1024:1536])
        nc.gpsimd.dma_start(wqq_sb[c][:, :], wqkvT_d[rr[c], 128:512])
    for c in range(NT):
        nc.scalar.dma_start(x_sb[c][1][:, :], x_d[rr[c], 512:1024])
        nc.gpsimd.dma_start(wqk_sb[c][:, :], wqkvT_d[rr[c], 640:1024])
        nc.sync.dma_start(x_sb[c][2][:, :], x_d[rr[c], 1024:1536])
    for c in range(NT):
        nc.sync.dma_start(x_sb[c][3][:, :], x_d[rr[c], 1536:2048])

    # V^T ones columns (index 64 of each head's 65-block): the PV
    # accumulator row 64 is the softmax denominator. (Engines require
    # 32-aligned partition offsets, so ones-at-0 + reading dims at rows
    # 1:65 is illegal; offset 64 for the den row is aligned.) Emitted
    # after the DMA triggers; first consumer (PV(0)) is at ~14us.
    for j in range(NJ):
        nc.vector.memset(
            vt1[j].rearrange("p (h e) -> p h e", e=65)[:, :, 64:65], 1.0
        )
    # [1,64] ones row: lhsT of the tail's K=1 broadcast matmuls
    ones64 = pers.tile([1, 64], BF16, tag="ones64", name="ones64")
    nc.vector.memset(ones64[:, :], 1.0)

    # ---- PE warm-up: dummy matmuls during the input-DMA window so the HAM
    # clock gate opens (1.2 -> 2.4 GHz) before the real work arrives. The
    # chain ends in a DMA to an internal DRAM scratch so DCE keeps it. ----
    # warm-up evacuation on ScalarE (idle until its first ACT at ~14us):
    # on VectorE it head-of-line blocked the q/k evacuation CASTs behind
    # the warm matmuls.
    wps = qkp.tile([128, 512], F32, tag="qkp", name="warm_ps")
    for w in range(NWARM):
        nc.tensor.matmul(
            wps[:, :], lhsT=warm_sb[:, 0:128], rhs=warm_sb[:, :],
            start=True, stop=True,
        )
    nc.scalar.copy(warm_out[:, :], wps[:, :])
    nc.sync.dma_start(warm_scratch.ap()[:, :], warm_out[:, :])

    def emit_qk_group(t, kind, n):
        """One projection psum group: q (kind=0) or k (kind=1) rows
        128t..128t+128 (heads 2t, 2t+1), l-chunk n. Lands directly in
        q_sb/k_sb (head 2t on partitions 0-63, head 2t+1 on 64-127)."""
        dst = (q_sb, k_sb)[kind][t]
        if t == 0:
            w_sb, lo = (wqq0_sb, wqk0_sb)[kind], 0
        else:
            w_sb, lo = (wqq_sb, wqk_sb)[kind], 128 * (t - 1)
        ps = qkp.tile([128, 512], F32, tag="qkp", name=f"qk_ps_{kind}_{t}_{n}")
        for c in range(NT):
            nc.tensor.matmul(
                ps[:, :],
                lhsT=w_sb[c][:, lo : lo + 128],
                rhs=x_sb[c][n][:, :],
                start=(c == 0),
                stop=(c == NT - 1),
            )
        nc.vector.tensor_copy(dst[:, 512 * n : 512 * (n + 1)], ps[:, :])

    def emit_vt(jt):
        """V^T tile for key-block jt: [128 keys, 8 heads x (ones + 64 dims)]."""
        ps = qkp.tile([128, 512], F32, tag="qkp", name=f"vt_ps_{jt}")
        for c in range(NT):
            nc.tensor.matmul(
                ps[:, :],
                lhsT=x_sb[c][jt // 4][:, 128 * (jt % 4) : 128 * (jt % 4 + 1)],
                rhs=wqv_sb[c][:, :],
                start=(c == 0),
                stop=(c == NT - 1),
            )
        vv = vt1[jt].rearrange("p (h e) -> p h e", e=65)
        nc.vector.tensor_copy(vv[:, :, 0:64], ps.rearrange("p (h d) -> p h d", d=64))

    def emit_st_for(t, ic, jt):
        islice = slice(512 * ic, 512 * ic + 512)
        jslice = slice(128 * jt, 128 * (jt + 1))
        st = stp.tile([128, 1024], F32, tag="st", name=f"st_{t}_{ic}_{jt}")
        # the two K=64 matmuls run concurrently (PE row groups 0-1 / 2-3)
        nc.tensor.matmul(
            st[:, 0:512], lhsT=k_sb[t][0:64, jslice], rhs=q_sb[t][0:64, islice],
            start=True, stop=True,
        )
        nc.tensor.matmul(
            st[:, 512:1024], lhsT=k_sb[t][64:128, jslice],
            rhs=q_sb[t][64:128, islice],
            start=True, stop=True,
        )
        return st

    def emit_pair(t, ic, interleave, vt_jit=False, carry_in=None, next_ti=None,
                  prev_norm=None, pe_rb=False):
        """Attention for head pair (2t, 2t+1), i-chunk ic (512 queries).
        S^T is emitted with a lead of 2: iterations NJ-2/NJ-1 emit the NEXT
        pair's S^T(0)/S^T(1) (returned as carry for the next emit_pair), so
        at a pair boundary ScalarE's exp never waits behind the previous
        pair's last PVs in the in-order PE stream. `prev_norm` is the
        previous pair's deferred normalization, woven into this loop:
        o2u copies at jt=0 (frees the shared ot psum early, so this pair's
        PV(0) is not gated on the rec/mul chain), recip+broadcast at jt=1,
        normalize muls at jt=2. Returns (carry_out, norm_closures)."""
        h0, h1 = 2 * t, 2 * t + 1
        ib = 512 * ic
        islice = slice(ib, ib + 512)
        ot0 = otp.tile([65, 512], F32, tag="ot0", name=f"ot0_{t}_{ic}")
        ot1 = otp.tile([65, 512], F32, tag="ot1", name=f"ot1_{t}_{ic}")

        def emit_st(jt):
            return emit_st_for(t, ic, jt)

        pv_done = [0]

        def emit_pv(jt, pt):
            pv_done[0] += 1
            stop = pv_done[0] == NJ
            vt = vt1[jt]
            nc.tensor.matmul(
                ot0[:, :], lhsT=vt[:, 65 * h0 : 65 * h0 + 65], rhs=pt[:, 0:512],
                start=(jt == 0), stop=stop,
            )
            nc.tensor.matmul(
                ot1[:, :], lhsT=vt[:, 65 * h1 : 65 * h1 + 65], rhs=pt[:, 512:1024],
                start=(jt == 0), stop=stop,
            )

        carry_out = []
        if carry_in is not None:
            sts = {0: carry_in[0], 1: carry_in[1]}
        else:
            sts = {0: emit_st(0), 1: emit_st(1)}
        for jt in range(NJ):
            pt = ptp.tile([128, 1024], BF16, tag="pt", name=f"pt_{t}_{ic}_{jt}")
            st_t = sts.pop(jt)
            # column-split exp: ScalarE cols 0:cs_jt, VectorE's 2-op poly
            # cols cs_jt:1024. The first NSC jts are all-ScalarE so VectorE
            # is free for the previous pair's deferred normalization.
            cs_jt = 1024 if jt < NSC else CS2
            if cs_jt > 0:
                nc.scalar.activation(
                    pt[:, 0:cs_jt], st_t[:, 0:cs_jt], AF.Exp, scale=16.0
                )
            if cs_jt < 1024:
                cv = 1024 - cs_jt
                p1 = scrp.tile([128, cv], F32, tag="p1", name=f"p1_{t}_{ic}_{jt}")
                nc.vector._custom_dve(
                    EXP16_POLY, out=p1[:, :], in0=st_t[:, cs_jt:1024],
                    s0=EXP_C[0], s1=EXP_C[1], imm2=EXP_C[2],
                )
                nc.vector._custom_dve(
                    POW16, out=pt[:, cs_jt:1024], in0=p1[:, :]
                )
            # deferred normalization of the PREVIOUS pair: o2u/den/recip
            # chain at jt=0 (VectorE has no poly work then), the K=1
            # reciprocal-broadcast matmuls at jt=3 (late enough that the
            # in-order PE never waits on the VectorE recip/cast chain),
            # the muls at jt=8/9 (an earlier mul emission head-of-line
            # blocks the VectorE queue and the polys behind it).
            if prev_norm is not None and jt in (0, 3, 8, 9):
                prev_norm[{0: 0, 3: 1, 8: 2, 9: 3}[jt]]()
            # lead-2 S^T: in the PE's in-order stream the matmuls feeding
            # exp(jt+2) run before PV(jt), so the exp chain never waits on
            # a PV (which itself waits on exp output / ot psum free).
            if jt + 2 < NJ:
                sts[jt + 2] = emit_st(jt + 2)
            elif next_ti is not None:
                carry_out.append(emit_st_for(next_ti[0], next_ti[1], jt + 2 - NJ))
            # V^T tiles emitted in-loop so they never gate the first exp;
            # >=2-iteration lead keeps their copies off PV's critical path
            if vt_jit:
                if jt == 0:
                    emit_vt(0)
                    emit_vt(1)
                    emit_vt(2)
                elif jt + 2 < NJ:
                    emit_vt(jt + 2)
            emit_pv(jt, pt)
            # explicitly-scheduled independent PE work. A pair's OWN kg(t,n)
            # closure must be emitted before loop jt=4n-2 (where the lead-2
            # S^T first reads that k chunk) or the in-order PE stream
            # deadlocks; everything else fills PE slack mid-loop.
            for fn in interleave.get(jt, ()):
                fn()
        # deferred softmax normalization closures (run during the NEXT
        # pair's loop, or immediately for the last pair): row 64 of ot is
        # the denominator; it is staged to partition 0 (an aligned [64:65]
        # read) because reciprocal_approx_fast mis-reads non-zero partition
        # offsets on silicon.
        o2us, rbs, rb16s = {}, {}, {}
        last = next_ti is None

        def norm_phase0():
            # o2u copies first (frees the shared ot psum for the next
            # pair's PV(0)), then the den->recip chain per head. Mid-loop
            # the reciprocal row is partition-broadcast on GpSimd; for the
            # LAST pair the broadcast is a K=1 ones-row matmul into a free
            # stp psum instead - the PE is idle at the tail and the two
            # serial ~1.2us GpSimd broadcasts would sit on the critical
            # path to the final projections.
            rbps = stp.tile([128, 1024], F32, tag="st", name="tail_rb_ps") \
                if last else None
            if not last:
                # both o2u copies FIRST: they free the shared ot psum that
                # this pair's PV(0) accumulates into
                for p, ot in ((0, ot0), (1, ot1)):
                    hh = 2 * t + p
                    o2u = smp.tile([65, 512], F32, tag=f"o2u{p}",
                                   name=f"o2u_{hh}_{ic}")
                    nc.vector.tensor_copy(o2u[:, :], ot[:, :])
                    o2us[p] = o2u
            for p, ot in ((0, ot0), (1, ot1)):
                hh = 2 * t + p
                # for the last pair the h1 copies run on the (idle) ScalarE
                # so the two heads' chains overlap instead of serializing
                # on the VectorE queue
                cp = nc.scalar.copy if (last and p == 1) else nc.vector.tensor_copy
                if last:
                    o2u = smp.tile([65, 512], F32, tag=f"o2u{p}",
                                   name=f"o2u_{hh}_{ic}")
                    cp(o2u[:, :], ot[:, :])
                    o2us[p] = o2u
                den = smp.tile([1, 512], F32, tag=f"den{p}", name=f"den_{hh}_{ic}")
                cp(den[:, :], o2us[p][64:65, :])
                rec = smp.tile([1, 512], F32, tag=f"rec{p}", name=f"rec_{hh}_{ic}")
                nc.vector.reciprocal_approx_fast(rec[:, :], den[:, :])
                if last or pe_rb:
                    rb16 = smp.tile([1, 512], BF16, tag=f"rb16{p}",
                                    name=f"rb16_{hh}_{ic}")
                    nc.vector.tensor_copy(rb16[:, :], rec[:, :])
                    rb16s[p] = rb16
                    if last:
                        nc.tensor.matmul(
                            rbps[64 * p : 64 * p + 64, 0:512],
                            lhsT=ones64[0:1, :], rhs=rb16[0:1, :],
                            start=True, stop=True,
                        )
                        rbs[p] = rbps[64 * p : 64 * p + 64, 0:512]
                else:
                    rb = smp.tile([64, 512], F32, tag=f"rb{p}", name=f"rb_{hh}_{ic}")
                    nc.gpsimd.partition_broadcast(rb[:, :], rec[:, :])
                    rbs[p] = rb[:, :]

        def norm_phase0b():
            # K=1 ones-row broadcast matmuls into a qkp psum (live jt3..9;
            # the qg group holds the other buffer jt4-7, pj reuses this one
            # at jt11 after the muls read it - exactly 2 bufs). Replaces
            # the serial ~1.2us GpSimd broadcasts whose round-trip through
            # the backlogged VectorE recip chain gated the second mul.
            if not pe_rb or last:
                return
            rbq = qkp.tile([128, 512], F32, tag="qkp", name=f"rb_ps_{t}_{ic}")
            for p in (0, 1):
                nc.tensor.matmul(
                    rbq[64 * p : 64 * p + 64, 0:512],
                    lhsT=ones64[0:1, :], rhs=rb16s[p][0:1, :],
                    start=True, stop=True,
                )
                rbs[p] = rbq[64 * p : 64 * p + 64, 0:512]

        def norm_mul(p):
            def f():
                nc.vector.tensor_mul(
                    o2[t][64 * p : 64 * p + 64, islice], o2us[p][0:64, :], rbs[p]
                )
            return f

        return carry_out, [norm_phase0, norm_phase0b, norm_mul(0), norm_mul(1)]

    held_proj = {}

    def emit_proj_group(o, n, c_lo=0, bias_on_scalar=False, dma_engine=None):
        if c_lo == 0:
            ps = qkp.tile([128, 512], F32, tag="qkp", name=f"y_ps_{o}_{n}")
        else:
            ps = held_proj.pop((o, n))
        for c in range(c_lo, NT):
            nc.tensor.matmul(
                ps[:, :],
                lhsT=wo_sb[c][:, 128 * o : 128 * (o + 1)],
                rhs=o2[c][:, 512 * n : 512 * (n + 1)],
                start=(c == 0),
                stop=(c == NT - 1),
            )
        yt = ytp.tile([128, 512], F32, tag="yt", name=f"yt_{o}_{n}")
        if bias_on_scalar:
            nc.scalar.add(yt[:, :], ps[:, :], bias_sb[o][:, 0:1])
        else:
            nc.vector.tensor_scalar_add(yt[:, :], ps[:, :], bias_sb[o][:, 0:1])
        (dma_engine or nc.sync).dma_start(
            out_d[128 * o : 128 * (o + 1), 512 * n : 512 * (n + 1)], yt[:, :]
        )

    def emit_proj_partial(o, n, ps=None):
        """First 3 channel-tiles of proj group (o, n); the psum tile is held
        and finished by emit_proj_group(o, n, c_lo=3) once the last pair's
        output is ready. `ps` lets the caller donate a psum region (the stp
        pool is free after the last pair's loop)."""
        if ps is None:
            ps = qkp.tile([128, 512], F32, tag="qkp", name=f"y_ps_{o}_{n}")
        for c in range(3):
            nc.tensor.matmul(
                ps[:, :],
                lhsT=wo_sb[c][:, 128 * o : 128 * (o + 1)],
                rhs=o2[c][:, 512 * n : 512 * (n + 1)],
                start=(c == 0),
                stop=False,
            )
        held_proj[(o, n)] = ps

    # ---- emission schedule ----
    # pair 0's q (chunk 0) + full k projected up front; everything else is
    # interleaved just-in-time into earlier attention loops.
    emit_qk_group(0, 0, 0)
    emit_qk_group(0, 1, 0)

    # wo/bias loads off the critical startup path
    for c in range(NT):
        r = slice(128 * c, 128 * (c + 1))
        nc.sync.dma_start(wo_sb[c][:, :], woutT_d[r, :])
        nc.sync.dma_start(bias_sb[c][:, :], bias_d[r, :])

    # split interleaved groups: matmuls at one jt, the psum evacuation a
    # few jts later. An inline evacuation head-of-line blocks the VectorE
    # queue (it waits its matmuls, which sit behind PV(jt) in the in-order
    # PE stream) and delays every poly behind it, which gates st-buffer
    # recycling -> S^T -> exp (measured 1.3us/pair at jt6). The evac jt
    # must stay (a) before the first S^T/carry emission that reads the
    # chunk and (b) close enough that at most 2 qkp tiles are ever held.
    pending = {}

    def _qk_mm(t, kind, n):
        if t == 0:
            w_sb, lo = (wqq0_sb, wqk0_sb)[kind], 0
        else:
            w_sb, lo = (wqq_sb, wqk_sb)[kind], 128 * (t - 1)
        ps = qkp.tile([128, 512], F32, tag="qkp", name=f"qk_ps_{kind}_{t}_{n}")
        for c in range(NT):
            nc.tensor.matmul(
                ps[:, :], lhsT=w_sb[c][:, lo : lo + 128], rhs=x_sb[c][n][:, :],
                start=(c == 0), stop=(c == NT - 1),
            )
        pending[(kind, t, n)] = ps

    def kgm(t, n):
        return lambda: _qk_mm(t, 1, n)

    def qgm(t, n):
        return lambda: _qk_mm(t, 0, n)

    def qke(kind, t, n):
        def f():
            ps = pending.pop((kind, t, n))
            dst = (q_sb, k_sb)[kind][t]
            nc.vector.tensor_copy(dst[:, 512 * n : 512 * (n + 1)], ps[:, :])
        return f

    def pjm(o, n):
        def f():
            ps = qkp.tile([128, 512], F32, tag="qkp", name=f"y_ps_{o}_{n}")
            for c in range(NT):
                nc.tensor.matmul(
                    ps[:, :],
                    lhsT=wo_sb[c][:, 128 * o : 128 * (o + 1)],
                    rhs=o2[c][:, 512 * n : 512 * (n + 1)],
                    start=(c == 0), stop=(c == NT - 1),
                )
            pending[('y', o, n)] = ps
        return f

    def pje(o, n):
        def f():
            ps = pending.pop(('y', o, n))
            yt = ytp.tile([128, 512], F32, tag="yt", name=f"yt_{o}_{n}")
            nc.vector.tensor_scalar_add(yt[:, :], ps[:, :], bias_sb[o][:, 0:1])
            nc.sync.dma_start(
                out_d[128 * o : 128 * (o + 1), 512 * n : 512 * (n + 1)], yt[:, :]
            )
        return f

    def kg(t, n):
        return lambda: emit_qk_group(t, 1, n)

    # pair t's q chunk for pass ic must be emitted BEFORE the previous
    # pair's loop jt=14 (which emits (ic,t)'s S^T(0) with the lead-2
    # carry); proj chunk n fires during pass n+1 (its last normalize muls
    # land at that pass's jt=8/9). A pair's OWN k chunk n is first read by
    # the lead-2 S^T at loop jt=4n-2, so kg(t,n) sits at jt 4n-3 or
    # earlier. Values: {loop_jt: [closures emitted after that jt's PV]}.
    inter = {
        (0, 0): {1: [kg(0, 1)], 4: [kgm(0, 2)], 5: [qke(1, 0, 2), qgm(1, 0)],
                 6: [qke(0, 1, 0)], 7: [kgm(1, 0)], 8: [qke(1, 1, 0),
                 kgm(0, 3)], 9: [qke(1, 0, 3)]},
        (0, 1): {1: [kg(1, 1)], 4: [kgm(1, 2)], 5: [qke(1, 1, 2), qgm(2, 0)],
                 6: [qke(0, 2, 0)], 7: [kgm(2, 0)], 8: [qke(1, 2, 0),
                 kgm(1, 3)], 9: [qke(1, 1, 3)]},
        (0, 2): {1: [kg(2, 1)], 4: [kgm(2, 2)], 5: [qke(1, 2, 2), qgm(3, 0)],
                 6: [qke(0, 3, 0)], 7: [kgm(3, 0)], 8: [qke(1, 3, 0),
                 kgm(2, 3)], 9: [qke(1, 2, 3)], 10: [qgm(0, 1)],
                 11: [qke(0, 0, 1)]},
        (0, 3): {1: [kg(3, 1)], 4: [kgm(3, 2)], 5: [qke(1, 3, 2)],
                 8: [kgm(3, 3)], 9: [qke(1, 3, 3)], 10: [qgm(1, 1)],
                 11: [qke(0, 1, 1)], 12: [qgm(2, 1)], 13: [qke(0, 2, 1),
                 qgm(3, 1)], 14: [qke(0, 3, 1)]},
        (1, 0): {4: [qgm(0, 2)], 7: [qke(0, 0, 2)], 11: [pjm(0, 0)], 12: [pje(0, 0)]},
        (1, 1): {4: [qgm(1, 2)], 7: [qke(0, 1, 2)], 11: [pjm(1, 0)], 12: [pje(1, 0)]},
        (1, 2): {4: [qgm(2, 2)], 7: [qke(0, 2, 2)], 11: [pjm(2, 0)], 12: [pje(2, 0)]},
        (1, 3): {4: [qgm(3, 2)], 7: [qke(0, 3, 2)], 11: [pjm(3, 0)], 12: [pje(3, 0)]},
        (2, 0): {4: [qgm(0, 3)], 7: [qke(0, 0, 3)], 11: [pjm(0, 1)], 12: [pje(0, 1)]},
        (2, 1): {4: [qgm(1, 3)], 7: [qke(0, 1, 3)], 11: [pjm(1, 1)], 12: [pje(1, 1)]},
        (2, 2): {4: [qgm(2, 3)], 7: [qke(0, 2, 3)], 11: [pjm(2, 1)], 12: [pje(2, 1)]},
        (2, 3): {4: [qgm(3, 3)], 7: [qke(0, 3, 3)], 11: [pjm(3, 1)], 12: [pje(3, 1)]},
        # pj emissions sit at jt>=11: the previous pair's normalize muls
        # (which write the o2 slices they read) are emitted at jt=8/9.
        (3, 0): {11: [pjm(0, 2)], 12: [pje(0, 2)]},
        (3, 1): {11: [pjm(1, 2)], 12: [pje(1, 2)]},
        (3, 2): {11: [pjm(2, 2)], 12: [pje(2, 2)], 13: [pjm(3, 2)],
                 14: [pje(3, 2)]},
        (3, 3): {11: [lambda: emit_proj_partial(0, 3)],
                 13: [lambda: emit_proj_partial(1, 3)]},
    }
    seq = [(ic, t) for ic in range(4) for t in range(NT)]
    carry = None
    norm = None
    for i, (ic, t) in enumerate(seq):
        nxt = seq[i + 1] if i + 1 < len(seq) else None
        carry, norm = emit_pair(
            t, ic, inter.get((ic, t), {}),
            vt_jit=(ic == 0 and t == 0),
            carry_in=carry,
            next_ti=(nxt[1], nxt[0]) if nxt else None,
            prev_norm=norm,
            # PE-matmul reciprocal broadcast only when the norm's HOST pair
            # (i+1) is in pass>=1: pass-0 hosts have no free qkp slot
            pe_rb=(i >= 3),
        )
    # last pair: run its deferred normalization now; the 2,3 proj partials
    # borrow the stp psum (free once the last exp pair drained).
    tail_ps = stp.tile([128, 1024], F32, tag="st", name="tail_ps")
    norm[0]()
    norm[1]()
    emit_proj_partial(2, 3, ps=tail_ps[:, 0:512])
    emit_proj_partial(3, 3, ps=tail_ps[:, 512:1024])
    norm[2]()
    norm[3]()
    # final groups: bias on the idle ScalarE, output DMAs spread over three
    # trigger queues so the last transfers overlap
    emit_proj_group(0, 3, c_lo=3, bias_on_scalar=True)
    emit_proj_group(1, 3, c_lo=3, bias_on_scalar=True, dma_engine=nc.gpsimd)
    emit_proj_group(2, 3, c_lo=3, bias_on_scalar=True, dma_engine=nc.scalar)
    emit_proj_group(3, 3, c_lo=3, bias_on_scalar=True, dma_engine=nc.gpsimd)
    ctx.close()


_COMPILED = None


def _build(debug=False):
    nc = bacc.Bacc(
        "TRN2", target_bir_lowering=False, debug=debug, num_devices=NCORES
    )
    x_d = nc.dram_tensor("x", [C, L], BF16, kind="ExternalInput").ap()
    wqkvT_d = nc.dram_tensor("wqkvT", [C, 3 * HID], BF16, kind="ExternalInput").ap()
    woutT_d = nc.dram_tensor("woutT", [HID, C], BF16, kind="ExternalInput").ap()
    bias_d = nc.dram_tensor("bias", [C, 1], F32, kind="ExternalInput").ap()
    out_d = nc.dram_tensor("out", [C, L], F32, kind="ExternalOutput").ap()
    with tile.TileContext(nc) as tc:
        build_kernel(tc, out_d, x_d, wqkvT_d, woutT_d, bias_d)
    nc.compile()
    return nc


def _get_compiled():
    global _COMPILED
    if _COMPILED is None:
        _COMPILED = _build(debug=False)
    return _COMPILED


def make_in_maps(x, w_qkv, w_out, b_out):
    xb = np.asarray(x, dtype=np.float32).astype(ml_dtypes.bfloat16)
    wq_f = np.asarray(w_qkv, dtype=np.float32).T.copy()
    wq_f[:, 0:HID] *= SCALE / 16.0  # exp scale folded into the q projection
    wqkvT = np.ascontiguousarray(wq_f.astype(ml_dtypes.bfloat16))
    woutT = np.ascontiguousarray(
        np.asarray(w_out, dtype=np.float32).T.astype(ml_dtypes.bfloat16)
    )
    bias = np.ascontiguousarray(np.asarray(b_out, dtype=np.float32).reshape(C, 1))
    return [
        {
            "x": np.ascontiguousarray(xb[b]),
            "wqkvT": wqkvT,
            "woutT": woutT,
            "bias": bias,
        }
        for b in range(B)
    ]


LAST_RESULTS = None


def _install_ntff_hook():
    """Provide antenv.axon_hooks (absent from this image) so trace=True works."""
    import types

    try:
        from antenv.axon_hooks import get_axon_ntff_profile_hook  # noqa: F401

        return
    except ImportError:
        pass
    sys.path.insert(0, "/root/.axon_site")
    from trn_agent_boot.trn_boot import _ntff_profile_via_ctypes

    hook = _ntff_profile_via_ctypes("/opt/axon/libaxon_pjrt.so")
    import antenv

    mod = types.ModuleType("antenv.axon_hooks")
    mod._hook = hook
    mod.get_axon_ntff_profile_hook = lambda: mod._hook
    mod.set_axon_ntff_profile_hook = lambda h: setattr(mod, "_hook", h)
    sys.modules["antenv.axon_hooks"] = mod
    antenv.axon_hooks = mod
    # artifact upload has no egress in this container - make it a no-op
    bass_utils.upload_artifacts = lambda tmpdir: tmpdir


def kernel(x, w_qkv, w_out, b_out):
    global LAST_RESULTS
    nc = _get_compiled()
    in_maps = make_in_maps(x, w_qkv, w_out, b_out)
    trace = bool(int(os.environ.get("KERNEL_TRACE", "0")))
    if trace:
        _install_ntff_hook()
    res = bass_utils.run_bass_kernel_spmd(
        nc, in_maps, core_ids=list(range(NCORES)), trace=trace
    )
    LAST_RESULTS = res
    out = np.stack([np.asarray(res.results[b]["out"]) for b in range(B)])
    return out.astype(np.float32)
